# revision 4
# baseline (speedup 1.0000x reference)
"""Trainium2 Bass kernel for ConvBottleneckBlock (LN -> conv1d 1->32 k3 -> gelu -> conv1d 32->1 k3 -> residual).

Data parallel over batch: 8 cores x 256 rows. Per core (R=256, D=4096, C=32):

  Phase 1: DMA x row-major [2 x (128, 4096)], LN stats via DVE reduce,
           normalize, PE-transpose into h_T tiles [32 x (128 pos, 256 rows)],
           fusing gamma/beta into the PSUM->SBUF copy (per-partition scalars
           in transposed layout).
  Phase 2: down conv: per 4-position block t, a banded stationary matrix
           Wd[t%32] [128,128] contracts the full h_T tile:
           psum[(c,f)=128, 256] = Wd.T @ h_T[t//32]; cross-tile halo handled
           by two edge matrices (E_lo/E_hi) accumulating from the
           neighboring h_T tile.  Groups of G blocks share one PSUM tile;
           one ACT gelu (bias=b_down per partition) per group -> g tiles.
           up conv per 128-position m-tile: 34 accumulating matmuls, one per
           contributing g block, with banded stationary Wu[bp+1] [128,128]
           (memset + start=False, full-tile M=128 outputs).
  Phase 3: PE-transpose up_T back to row-major, DVE-add into x tiles
           (residual; b_up folded into the finalize copy), DMA out.

All matmuls are [128,128] stationary x [128,256] moving: PE streams 256
columns/matmul, LDWEIGHTS (128 cols) pipelines underneath; gelu runs on ACT
at 1 elem/lane/cycle, which is the roofline for this op.
"""
import numpy as np

NCORES = 8
R = 256          # rows per core
D = 4096
C = 32
NT = 32          # h_T / m position tiles of 128
NB = 1024        # 4-position blocks
G = 6            # blocks per gelu group (3 PSUM banks)
NG = (NB + G - 1) // G
EPS = 1e-5

_CACHE = {}


def _emit(ctx, tc, nc, mybir, aps):
    f32 = mybir.dt.float32
    Alu = mybir.AluOpType
    Act = mybir.ActivationFunctionType
    X = mybir.AxisListType.X

    xin, wdall, wuall, bdown, bup, gammaT, betaT, ident, out = aps

    consts = ctx.enter_context(tc.tile_pool(name="consts", bufs=1))
    xpool = ctx.enter_context(tc.tile_pool(name="xsb", bufs=2))
    stage = ctx.enter_context(tc.tile_pool(name="stage", bufs=2))
    stats = ctx.enter_context(tc.tile_pool(name="stats", bufs=20))
    hTp = ctx.enter_context(tc.tile_pool(name="hT", bufs=NT))
    gpool = ctx.enter_context(tc.tile_pool(name="g", bufs=8))
    upTp = ctx.enter_context(tc.tile_pool(name="upT", bufs=6))
    psd = ctx.enter_context(tc.tile_pool(name="psd", bufs=2, space="PSUM"))
    pss = ctx.enter_context(tc.tile_pool(name="pss", bufs=2, space="PSUM"))

    def ld(ap, shape, tag):
        t = consts.tile(shape, f32, tag=tag, name=tag)
        nc.sync.dma_start(t[:], ap)
        return t

    wd_sb = ld(wdall, [128, 34 * 128], "wdall")   # 32 banded + E_lo + E_hi
    wu_sb = ld(wuall, [128, 34 * 128], "wuall")   # banded for bp=-1..32
    bdown_sb = ld(bdown, [128, 1], "bdown")
    bup_sb = ld(bup, [128, 1], "bup")
    gammaT_sb = ld(gammaT, [128, NT], "gammaT")
    betaT_sb = ld(betaT, [128, NT], "betaT")
    ident_sb = ld(ident, [128, 128], "ident")

    def wd(m):
        return wd_sb[:, 128 * m:128 * (m + 1)]

    def wu(m):
        return wu_sb[:, 128 * m:128 * (m + 1)]

    hT = [hTp.tile([128, R], f32, tag="hT", name="hT") for _ in range(NT)]
    x_sb = []

    # ---- Phase 1: load, layernorm, transpose ----
    for v in range(2):
        xt = xpool.tile([128, D], f32, tag="x", name="xt")
        nc.sync.dma_start(xt[:], xin[128 * v:128 * (v + 1), :])
        x_sb.append(xt)

        s = stage.tile([128, D], f32, tag="stage", name="s")
        ssum = stats.tile([128, 1], f32, tag="st", name="ssum")
        nc.vector.reduce_sum(ssum[:], xt[:], axis=X)
        nc.vector.tensor_mul(s[:], xt[:], xt[:])
        sqs = stats.tile([128, 1], f32, tag="st", name="sqs")
        nc.vector.reduce_sum(sqs[:], s[:], axis=X)
        mu = stats.tile([128, 1], f32, tag="st", name="mu")
        nc.vector.tensor_scalar_mul(mu[:], ssum[:], 1.0 / D)
        e2 = stats.tile([128, 1], f32, tag="st", name="e2")
        nc.vector.tensor_scalar(e2[:], sqs[:], 1.0 / D, EPS, Alu.mult, Alu.add)
        mu2 = stats.tile([128, 1], f32, tag="st", name="mu2")
        nc.vector.tensor_mul(mu2[:], mu[:], mu[:])
        vpe = stats.tile([128, 1], f32, tag="st", name="vpe")
        nc.vector.tensor_sub(vpe[:], e2[:], mu2[:])
        std = stats.tile([128, 1], f32, tag="st", name="std")
        nc.scalar.sqrt(std[:], vpe[:])
        inv = stats.tile([128, 1], f32, tag="st", name="inv")
        nc.vector.reciprocal(inv[:], std[:])
        # s <- (x - mu) * inv_std   (gamma/beta applied post-transpose)
        nc.vector.tensor_scalar(s[:], xt[:], mu[:], inv[:], Alu.subtract, Alu.mult)

        for i in range(NT):
            pt = psd.tile([128, G * R], f32, tag="psd", name="pt")
            nc.tensor.transpose(pt[:, 0:128], s[:, 128 * i:128 * (i + 1)], ident_sb[:])
            nc.vector.tensor_scalar(
                hT[i][:, 128 * v:128 * (v + 1)], pt[:, 0:128],
                gammaT_sb[:, i:i + 1], betaT_sb[:, i:i + 1], Alu.mult, Alu.add)

    # ---- Phase 2: down conv + gelu groups, up conv m-tiles ----
    g_tiles = [None] * NG

    def emit_group(gi):
        b0 = gi * G
        nb = min(G, NB - b0)
        w = nb * R
        pg = psd.tile([128, G * R], f32, tag="psd", name="pg")
        for k in range(nb):
            t = b0 + k
            i, b = t // 32, t % 32
            oap = pg[:, R * k:R * (k + 1)]
            st = (k % 2 == 0)
            lp = (k % 2 == 1) or (k == nb - 1)
            nc.tensor.matmul(oap, wd(b), hT[i][:], start=st,
                             stop=lp and not (b == 31 and i < NT - 1))
            if b == 0 and i > 0:
                nc.tensor.matmul(oap, wd(32), hT[i - 1][:], start=False, stop=False,
                                 skip_group_check=True)
            elif b == 31 and i < NT - 1:
                nc.tensor.matmul(oap, wd(33), hT[i + 1][:], start=False, stop=lp)
        gt = gpool.tile([128, G * R], f32, tag="g", name="gt")
        nc.scalar.activation(gt[:, :w], pg[:, :w], Act.Gelu, bias=bdown_sb[:], scale=1.0)
        g_tiles[gi] = gt

    up_T = [None] * NT
    next_g = 0
    for i in range(NT):
        need = min((32 * i + 32) // G, NG - 1)
        while next_g <= need:
            emit_group(next_g)
            next_g += 1
        ups = pss.tile([128, R], f32, tag="pss", name="ups")
        nc.vector.memset(ups[:], 0.0)
        for t in range(32 * i - 1, 32 * i + 33):
            if t < 0 or t >= NB:
                continue
            bp = t - 32 * i
            gt = g_tiles[t // G]
            rhs = gt[:, R * (t % G):R * (t % G + 1)]
            nc.tensor.matmul(ups[:], wu(bp + 1), rhs, start=False, stop=False,
                             skip_group_check=True)
        ut = upTp.tile([128, R], f32, tag="upT", name="ut")
        nc.vector.tensor_scalar(ut[:], ups[:], bup_sb[:], None, Alu.add)
        up_T[i] = ut

        # ---- Phase 3 (interleaved): transpose back + residual ----
        for v in range(2):
            pt = psd.tile([128, G * R], f32, tag="psd", name="pt")
            nc.tensor.transpose(pt[:, 0:128], up_T[i][:, 128 * v:128 * (v + 1)],
                                ident_sb[:])
            nc.vector.tensor_add(x_sb[v][:, 128 * i:128 * (i + 1)],
                                 x_sb[v][:, 128 * i:128 * (i + 1)], pt[:, 0:128])

    for v in range(2):
        nc.sync.dma_start(out[128 * v:128 * (v + 1), :], x_sb[v][:])


def _build():
    from contextlib import ExitStack
    import concourse.tile as tile
    from concourse import bacc, mybir

    f32 = mybir.dt.float32
    nc = bacc.Bacc("TRN2", target_bir_lowering=False, debug=False,
                   enable_asserts=False, num_devices=NCORES)
    xin = nc.dram_tensor("x", [R, D], f32, kind="ExternalInput").ap()
    wdall = nc.dram_tensor("wdall", [128, 34 * 128], f32, kind="ExternalInput").ap()
    wuall = nc.dram_tensor("wuall", [128, 34 * 128], f32, kind="ExternalInput").ap()
    bdown = nc.dram_tensor("bdown", [128, 1], f32, kind="ExternalInput").ap()
    bup = nc.dram_tensor("bup", [128, 1], f32, kind="ExternalInput").ap()
    gammaT = nc.dram_tensor("gammaT", [128, NT], f32, kind="ExternalInput").ap()
    betaT = nc.dram_tensor("betaT", [128, NT], f32, kind="ExternalInput").ap()
    ident = nc.dram_tensor("ident", [128, 128], f32, kind="ExternalInput").ap()
    out = nc.dram_tensor("out", [R, D], f32, kind="ExternalOutput").ap()

    with tile.TileContext(nc) as tc, ExitStack() as ctx:
        _emit(ctx, tc, nc, mybir,
              (xin, wdall, wuall, bdown, bup, gammaT, betaT, ident, out))
    nc.compile()
    return nc


def get_nc():
    if "nc" not in _CACHE:
        _CACHE["nc"] = _build()
    return _CACHE["nc"]


def host_consts(gamma, beta, w_down, b_down, w_up, b_up):
    wdm = np.asarray(w_down, np.float32)[:, 0, :]   # [32, 3]
    wum = np.asarray(w_up, np.float32)[0]           # [32, 3]
    p = np.arange(128)
    cf = np.arange(128)
    c, f = cf // 4, cf % 4
    wdall = np.zeros((128, 34 * 128), np.float32)
    for m in range(32):
        k = p[:, None] - 4 * m + 1 - f[None, :]
        wdall[:, 128 * m:128 * (m + 1)] = np.where(
            (k >= 0) & (k < 3), wdm[np.broadcast_to(c, k.shape), np.clip(k, 0, 2)], 0.0)
    wdall[127, 128 * 32 + 4 * np.arange(32)] = wdm[:, 0]        # E_lo
    wdall[0, 128 * 33 + 4 * np.arange(32) + 3] = wdm[:, 2]      # E_hi
    wuall = np.zeros((128, 34 * 128), np.float32)
    m = np.arange(128)
    for bp in range(-1, 33):
        k = f[:, None] - m[None, :] + 4 * bp + 1
        wuall[:, 128 * (bp + 1):128 * (bp + 2)] = np.where(
            (k >= 0) & (k < 3), wum[np.broadcast_to(c[:, None], k.shape), np.clip(k, 0, 2)], 0.0)
    bband = np.repeat(np.asarray(b_down, np.float32), 4).reshape(128, 1)
    bupv = np.full((128, 1), np.asarray(b_up, np.float32)[0], np.float32)
    gT = np.ascontiguousarray(np.asarray(gamma, np.float32).reshape(NT, 128).T)
    bT = np.ascontiguousarray(np.asarray(beta, np.float32).reshape(NT, 128).T)
    I = np.eye(128, dtype=np.float32)
    return wdall, wuall, bband, bupv, gT, bT, I


LAST_EXEC_NS = None


def kernel(x, gamma, beta, w_down, b_down, w_up, b_up, _trace=False):
    global LAST_EXEC_NS
    from concourse.bass_utils import run_bass_kernel_spmd

    nc = get_nc()
    x = np.ascontiguousarray(np.asarray(x, np.float32))
    wdall, wuall, bband, bupv, gT, bT, I = host_consts(
        gamma, beta, w_down, b_down, w_up, b_up)
    in_maps = []
    for k in range(NCORES):
        in_maps.append({
            "x": x[R * k:R * (k + 1)],
            "wdall": wdall, "wuall": wuall, "bdown": bband, "bup": bupv,
            "gammaT": gT, "betaT": bT, "ident": I,
        })
    res = run_bass_kernel_spmd(nc, in_maps, list(range(NCORES)), trace=_trace)
    LAST_EXEC_NS = res.exec_time_ns
    return np.concatenate([res.results[k]["out"] for k in range(NCORES)], axis=0)


# revision 6
# speedup vs baseline: 2.4799x; 2.4799x over previous
"""Trainium2 Bass kernel for ConvBottleneckBlock (LN -> conv1d 1->32 k3 -> gelu -> conv1d 32->1 k3 -> residual).

Data parallel over batch: 8 cores x 256 rows. Per core (R=256, D=4096, C=32):

  Phase 1: DMA x row-major [2 x (128, 4096)], LN stats via DVE reduce,
           normalize, PE-transpose into h_T tiles [32 x (128 pos, 256 rows)],
           fusing gamma/beta into the PSUM->SBUF copy (per-partition scalars
           in transposed layout).
  Phase 2: down conv: per 4-position block t, a banded stationary matrix
           Wd[t%32] [128,128] contracts the full h_T tile:
           psum[(c,f)=128, 256] = Wd.T @ h_T[t//32]; cross-tile halo handled
           by two edge matrices (E_lo/E_hi) accumulating from the
           neighboring h_T tile.  Groups of G blocks share one PSUM tile;
           one ACT gelu (bias=b_down per partition) per group -> g tiles.
           up conv per 128-position m-tile: 34 accumulating matmuls, one per
           contributing g block, with banded stationary Wu[bp+1] [128,128]
           (memset + start=False, full-tile M=128 outputs).
  Phase 3: PE-transpose up_T back to row-major, DVE-add into x tiles
           (residual; b_up folded into the finalize copy), DMA out.

All matmuls are [128,128] stationary x [128,256] moving: PE streams 256
columns/matmul, LDWEIGHTS (128 cols) pipelines underneath; gelu runs on ACT
at 1 elem/lane/cycle, which is the roofline for this op.
"""
import numpy as np

NCORES = 8
R = 256          # rows per core
D = 4096
C = 32
NT = 32          # h_T / m position tiles of 128
NB = 1024        # 4-position blocks
G = 6            # blocks per gelu group (3 PSUM banks)
NG = (NB + G - 1) // G
EPS = 1e-5

_CACHE = {}


def _emit(ctx, tc, nc, mybir, aps):
    f32 = mybir.dt.float32
    bf16 = mybir.dt.bfloat16
    Alu = mybir.AluOpType
    Act = mybir.ActivationFunctionType
    X = mybir.AxisListType.X

    xin, wdall, wuall, bdown, bup, gammaT, betaT, ident, out = aps

    consts = ctx.enter_context(tc.tile_pool(name="consts", bufs=1))
    xpool = ctx.enter_context(tc.tile_pool(name="xsb", bufs=2))
    stage = ctx.enter_context(tc.tile_pool(name="stage", bufs=2))
    stats = ctx.enter_context(tc.tile_pool(name="stats", bufs=20))
    hTp = ctx.enter_context(tc.tile_pool(name="hT", bufs=NT))
    gpool = ctx.enter_context(tc.tile_pool(name="g", bufs=8))
    upTp = ctx.enter_context(tc.tile_pool(name="upT", bufs=6))
    psd = ctx.enter_context(tc.tile_pool(name="psd", bufs=2, space="PSUM"))
    pss = ctx.enter_context(tc.tile_pool(name="pss", bufs=2, space="PSUM"))

    def ld(ap, shape, tag, dt=f32):
        t = consts.tile(shape, dt, tag=tag, name=tag)
        nc.sync.dma_start(t[:], ap)
        return t

    wd_sb = ld(wdall, [128, 34 * 128], "wdall", bf16)   # 32 banded + E_lo + E_hi
    wu_sb = ld(wuall, [128, 34 * 128], "wuall", bf16)   # banded for bp=-1..32
    bdown_sb = ld(bdown, [128, 1], "bdown")
    bup_sb = ld(bup, [128, 1], "bup")
    gammaT_sb = ld(gammaT, [128, NT], "gammaT")
    betaT_sb = ld(betaT, [128, NT], "betaT")
    ident_sb = ld(ident, [128, 128], "ident", bf16)

    def wd(m):
        return wd_sb[:, 128 * m:128 * (m + 1)]

    def wu(m):
        return wu_sb[:, 128 * m:128 * (m + 1)]

    hT = [hTp.tile([128, R], bf16, tag="hT", name="hT") for _ in range(NT)]
    x_sb = []

    # ---- Phase 1: load, layernorm, transpose ----
    for v in range(2):
        xt = xpool.tile([128, D], f32, tag="x", name="xt")
        nc.sync.dma_start(xt[:], xin[128 * v:128 * (v + 1), :])
        x_sb.append(xt)

        s = stage.tile([128, D], bf16, tag="stage", name="s")
        sq = stage.tile([128, D], f32, tag="sq", name="sq")
        ssum = stats.tile([128, 1], f32, tag="st", name="ssum")
        nc.vector.reduce_sum(ssum[:], xt[:], axis=X)
        nc.vector.tensor_mul(sq[:], xt[:], xt[:])
        sqs = stats.tile([128, 1], f32, tag="st", name="sqs")
        nc.vector.reduce_sum(sqs[:], sq[:], axis=X)
        mu = stats.tile([128, 1], f32, tag="st", name="mu")
        nc.vector.tensor_scalar_mul(mu[:], ssum[:], 1.0 / D)
        e2 = stats.tile([128, 1], f32, tag="st", name="e2")
        nc.vector.tensor_scalar(e2[:], sqs[:], 1.0 / D, EPS, Alu.mult, Alu.add)
        mu2 = stats.tile([128, 1], f32, tag="st", name="mu2")
        nc.vector.tensor_mul(mu2[:], mu[:], mu[:])
        vpe = stats.tile([128, 1], f32, tag="st", name="vpe")
        nc.vector.tensor_sub(vpe[:], e2[:], mu2[:])
        std = stats.tile([128, 1], f32, tag="st", name="std")
        nc.scalar.sqrt(std[:], vpe[:])
        inv = stats.tile([128, 1], f32, tag="st", name="inv")
        nc.vector.reciprocal(inv[:], std[:])
        # s <- (x - mu) * inv_std   (gamma/beta applied post-transpose)
        nc.vector.tensor_scalar(s[:], xt[:], mu[:], inv[:], Alu.subtract, Alu.mult)

        for i in range(NT):
            pt = psd.tile([128, 2 * G * R], bf16, tag="psd", name="pt")
            nc.tensor.transpose(pt[:, 0:128], s[:, 128 * i:128 * (i + 1)], ident_sb[:])
            nc.vector.tensor_scalar(
                hT[i][:, 128 * v:128 * (v + 1)], pt[:, 0:128],
                gammaT_sb[:, i:i + 1], betaT_sb[:, i:i + 1], Alu.mult, Alu.add)

    # ---- Phase 2: down conv + gelu groups, up conv m-tiles ----
    g_tiles = [None] * NG

    def emit_group(gi):
        b0 = gi * G
        nb = min(G, NB - b0)
        w = nb * R
        pg = psd.tile([128, G * R], f32, tag="psd", name="pg")
        for k in range(nb):
            t = b0 + k
            i, b = t // 32, t % 32
            oap = pg[:, R * k:R * (k + 1)]
            st = (k % 2 == 0)
            lp = (k % 2 == 1) or (k == nb - 1)
            nc.tensor.matmul(oap, wd(b), hT[i][:], start=st,
                             stop=lp and not (b == 31 and i < NT - 1))
            if b == 0 and i > 0:
                nc.tensor.matmul(oap, wd(32), hT[i - 1][:], start=False, stop=False,
                                 skip_group_check=True)
            elif b == 31 and i < NT - 1:
                nc.tensor.matmul(oap, wd(33), hT[i + 1][:], start=False, stop=lp)
        gt = gpool.tile([128, G * R], bf16, tag="g", name="gt")
        nc.scalar.activation(gt[:, :w], pg[:, :w], Act.Gelu, bias=bdown_sb[:], scale=1.0)
        g_tiles[gi] = gt

    up_T = [None] * NT
    next_g = 0
    for i in range(NT):
        need = min((32 * i + 32) // G, NG - 1)
        while next_g <= need:
            emit_group(next_g)
            next_g += 1
        ups = pss.tile([128, R], f32, tag="pss", name="ups")
        nc.vector.memset(ups[:], 0.0)
        for t in range(32 * i - 1, 32 * i + 33):
            if t < 0 or t >= NB:
                continue
            bp = t - 32 * i
            gt = g_tiles[t // G]
            rhs = gt[:, R * (t % G):R * (t % G + 1)]
            nc.tensor.matmul(ups[:], wu(bp + 1), rhs, start=False, stop=False,
                             skip_group_check=True)
        ut = upTp.tile([128, R], bf16, tag="upT", name="ut")
        nc.vector.tensor_scalar(ut[:], ups[:], bup_sb[:], None, Alu.add)
        up_T[i] = ut

        # ---- Phase 3 (interleaved): transpose back + residual ----
        for v in range(2):
            pt = psd.tile([128, 2 * G * R], bf16, tag="psd", name="pt")
            nc.tensor.transpose(pt[:, 0:128], up_T[i][:, 128 * v:128 * (v + 1)],
                                ident_sb[:])
            nc.vector.tensor_add(x_sb[v][:, 128 * i:128 * (i + 1)],
                                 x_sb[v][:, 128 * i:128 * (i + 1)], pt[:, 0:128])

    for v in range(2):
        nc.sync.dma_start(out[128 * v:128 * (v + 1), :], x_sb[v][:])


def _build():
    from contextlib import ExitStack
    import concourse.tile as tile
    from concourse import bacc, mybir

    f32 = mybir.dt.float32
    nc = bacc.Bacc("TRN2", target_bir_lowering=False, debug=False,
                   enable_asserts=False, num_devices=NCORES)
    bf16 = mybir.dt.bfloat16
    xin = nc.dram_tensor("x", [R, D], f32, kind="ExternalInput").ap()
    wdall = nc.dram_tensor("wdall", [128, 34 * 128], bf16, kind="ExternalInput").ap()
    wuall = nc.dram_tensor("wuall", [128, 34 * 128], bf16, kind="ExternalInput").ap()
    bdown = nc.dram_tensor("bdown", [128, 1], f32, kind="ExternalInput").ap()
    bup = nc.dram_tensor("bup", [128, 1], f32, kind="ExternalInput").ap()
    gammaT = nc.dram_tensor("gammaT", [128, NT], f32, kind="ExternalInput").ap()
    betaT = nc.dram_tensor("betaT", [128, NT], f32, kind="ExternalInput").ap()
    ident = nc.dram_tensor("ident", [128, 128], bf16, kind="ExternalInput").ap()
    out = nc.dram_tensor("out", [R, D], f32, kind="ExternalOutput").ap()

    with tile.TileContext(nc) as tc, ExitStack() as ctx:
        _emit(ctx, tc, nc, mybir,
              (xin, wdall, wuall, bdown, bup, gammaT, betaT, ident, out))
    nc.compile()
    return nc


def get_nc():
    if "nc" not in _CACHE:
        _CACHE["nc"] = _build()
    return _CACHE["nc"]


def host_consts(gamma, beta, w_down, b_down, w_up, b_up):
    wdm = np.asarray(w_down, np.float32)[:, 0, :]   # [32, 3]
    wum = np.asarray(w_up, np.float32)[0]           # [32, 3]
    p = np.arange(128)
    cf = np.arange(128)
    c, f = cf // 4, cf % 4
    wdall = np.zeros((128, 34 * 128), np.float32)
    for m in range(32):
        k = p[:, None] - 4 * m + 1 - f[None, :]
        wdall[:, 128 * m:128 * (m + 1)] = np.where(
            (k >= 0) & (k < 3), wdm[np.broadcast_to(c, k.shape), np.clip(k, 0, 2)], 0.0)
    wdall[127, 128 * 32 + 4 * np.arange(32)] = wdm[:, 0]        # E_lo
    wdall[0, 128 * 33 + 4 * np.arange(32) + 3] = wdm[:, 2]      # E_hi
    wuall = np.zeros((128, 34 * 128), np.float32)
    m = np.arange(128)
    for bp in range(-1, 33):
        k = f[:, None] - m[None, :] + 4 * bp + 1
        wuall[:, 128 * (bp + 1):128 * (bp + 2)] = np.where(
            (k >= 0) & (k < 3), wum[np.broadcast_to(c[:, None], k.shape), np.clip(k, 0, 2)], 0.0)
    bband = np.repeat(np.asarray(b_down, np.float32), 4).reshape(128, 1)
    bupv = np.full((128, 1), np.asarray(b_up, np.float32)[0], np.float32)
    gT = np.ascontiguousarray(np.asarray(gamma, np.float32).reshape(NT, 128).T)
    bT = np.ascontiguousarray(np.asarray(beta, np.float32).reshape(NT, 128).T)
    import ml_dtypes
    bf = ml_dtypes.bfloat16
    I = np.eye(128, dtype=bf)
    return wdall.astype(bf), wuall.astype(bf), bband, bupv, gT, bT, I


LAST_EXEC_NS = None


def kernel(x, gamma, beta, w_down, b_down, w_up, b_up, _trace=False):
    global LAST_EXEC_NS
    from concourse.bass_utils import run_bass_kernel_spmd

    nc = get_nc()
    x = np.ascontiguousarray(np.asarray(x, np.float32))
    wdall, wuall, bband, bupv, gT, bT, I = host_consts(
        gamma, beta, w_down, b_down, w_up, b_up)
    in_maps = []
    for k in range(NCORES):
        in_maps.append({
            "x": x[R * k:R * (k + 1)],
            "wdall": wdall, "wuall": wuall, "bdown": bband, "bup": bupv,
            "gammaT": gT, "betaT": bT, "ident": I,
        })
    res = run_bass_kernel_spmd(nc, in_maps, list(range(NCORES)), trace=_trace)
    LAST_EXEC_NS = res.exec_time_ns
    return np.concatenate([res.results[k]["out"] for k in range(NCORES)], axis=0)
